# revision 1
# baseline (speedup 1.0000x reference)
"""Trainium2 Bass kernel for causal multi-head attention (v3: no collective).

Problem: B=2, C=2048, H=1024, 16 heads, head_dim=64, float32.
    qkv = x @ Wqkv.T + b ; causal softmax attention ; out = att @ Wo.T + b

Sharding over 8 NeuronCores: core c owns heads {2c, 2c+1} for BOTH batches
(tensor parallel over heads). Each core computes q/k/v for its heads over
all 4096 tokens, runs causal attention for all 8 (batch, q-quarter)
slices, and applies a PARTIAL output projection (contraction over its own
128 head-dims only). The host unshard step sums the 8 partial [4096, 1024]
outputs and adds Wo_b (tensor-parallel reduce folded into the gather).

Schedule: attention is ScalarE-exp-bound (exp at ~0.96 GHz is 2x slower
than the S^T+AV matmuls it feeds), so QKV tiles for batch 1 and the
per-slice Wo matmuls are drained from a filler queue between attention
kb-steps to keep the PE busy; AV runs one kb behind S^T/exp.

Layouts: all weights/x are host-packed so each SBUF tile is ONE dma with
2-8KB contiguous segments. v is re-tiled [t, d] via DMA-transpose (XBAR)
with a ones column per head so the AV matmul also emits softmax row sums.
"""
import math
from collections import deque

import numpy as np

C, H, NH, HD = 2048, 1024, 16, 64
B = 2
NCORES = 8

_cache = {}


def _build():
    import concourse.bass as bass
    import concourse.bacc as bacc
    import concourse.tile as tile
    import concourse.mybir as mybir

    dt = mybir.dt
    f32 = dt.float32
    f32r = dt.float32r
    bf16 = dt.bfloat16
    AF = mybir.ActivationFunctionType

    nc = bacc.Bacc("TRN2", target_bir_lowering=False, debug=False,
                   enable_asserts=True, num_devices=NCORES)

    def din(name, shape, d=f32):
        return nc.dram_tensor(name, shape, d, kind="ExternalInput").ap()

    xTb = din("xTb", [1024, 4096], bf16)      # [tt*128+p, hb*512+j]
    wqkb = din("wqkb", [128, 2048], bf16)     # [p, hb*256+ob*128+o]
    wvb = din("wvb", [128, 1024], bf16)       # [p, hb*128+o]
    qkb = din("qkb", [128, 2])                # bias cols: q, k
    vbcol = din("vbcol", [128, 1])            # v bias per out dim
    masks128 = din("masks128", [128, 128], bf16)
    ident = din("ident", [128, 128], bf16)
    ones1x64 = din("ones1x64", [1, 64], f32r)
    wob = din("wob", [128, 1024], bf16)       # Wo^T rows for my 128 dims
    y_out = nc.dram_tensor("y", [4096, 1024], bf16, kind="ExternalOutput").ap()

    with tile.TileContext(nc) as tc:
        ctx_lp = nc.allow_low_precision(
            reason="bf16 operands; all matmuls accumulate in f32 PSUM")
        ctx_lp.__enter__()
        with (
            tc.tile_pool(name="const", bufs=1) as const_pool,
            tc.tile_pool(name="persist", bufs=1) as persist,
            tc.tile_pool(name="qkvps", bufs=2, space="PSUM") as qkv_ps,
            tc.tile_pool(name="sps", bufs=2, space="PSUM") as s_ps,
            tc.tile_pool(name="avps", bufs=1, space="PSUM") as av_ps,
            tc.tile_pool(name="xt", bufs=4) as xt_pool,
            tc.tile_pool(name="psb", bufs=8) as p_pool,
            tc.tile_pool(name="attsb", bufs=8) as att_pool,
            tc.tile_pool(name="recsb", bufs=2) as rec_pool,
            tc.tile_pool(name="ysb", bufs=3) as y_pool,
        ):
            # -------- weights + first inputs (critical path order)
            wqk_sb = const_pool.tile([128, 2048], bf16, tag="wqk")
            nc.sync.dma_start(wqk_sb[:], wqkb)
            qkb_sb = const_pool.tile([128, 2], f32, tag="qkb")
            nc.sync.dma_start(qkb_sb[:], qkb)
            # preload the exp table set during startup DMA wait
            dummy = rec_pool.tile([1, 2], f32, tag="dummy", name="dummy")
            nc.scalar.activation(dummy[:], qkb_sb[0:1, 0:2], AF.Exp)
            wv_sb = const_pool.tile([128, 1024], bf16, tag="wv")
            nc.sync.dma_start(wv_sb[:], wvb)
            vb_sb = const_pool.tile([128, 1], f32, tag="vb")
            nc.sync.dma_start(vb_sb[:], vbcol)

            # constants first needed later (gpsimd queue, off critical path)
            masks_sb = const_pool.tile([128, 128], bf16, tag="masks")
            nc.gpsimd.dma_start(masks_sb[:], masks128)
            ident_sb = const_pool.tile([128, 128], bf16, tag="ident")
            nc.sync.dma_start(ident_sb[:], ident)
            ones_sb = const_pool.tile([1, 64], f32r, tag="ones")
            nc.gpsimd.dma_start(ones_sb[:], ones1x64)
            wob_sb = const_pool.tile([128, 1024], bf16, tag="wob")
            nc.gpsimd.dma_start(wob_sb[:], wob)

            # -------- persistent activations
            qT_sb = persist.tile([128, 4096], bf16, tag="qT")
            kT_sb = persist.tile([128, 4096], bf16, tag="kT")
            vT_sb = persist.tile([128, 4096], bf16, tag="vT")
            # slot layout (width 256): [vA 0:64 | onesA 64 | pad | vB 128:192
            # | onesB 192 | pad] -- DMA-transpose dests need 64-col alignment
            v_sb = persist.tile([128, 32 * 256], bf16, tag="v")
            ones_view = v_sb[:].rearrange("p (s h e) -> p s h e", s=32, h=2, e=128)
            nc.gpsimd.memset(ones_view[:, :, :, 64], 1.0)

            # ---------------- filler queue ----------------
            warm_ps = [None]  # current avA tile; spare partitions host junk
            filler = deque()  # (pe_cost_ns, emit_fn, tag)
            emitted_tags = set()

            def drain(budget_ns):
                while budget_ns > 0 and filler:
                    cost, fn, tag = filler.popleft()
                    fn()
                    if tag is not None:
                        emitted_tags.add(tag)
                    budget_ns -= cost


            def drain_all():
                drain(float("inf"))

            def drain_until(tag):
                while tag not in emitted_tags and filler:
                    cost, fn, t = filler.popleft()
                    fn()
                    if t is not None:
                        emitted_tags.add(t)

            # ---------------- building blocks ----------------
            def enqueue_qkv(tt, split=False):
                """QKV projection for one 512-token tile; xt DMA issued now
                (prefetch), matmul groups go on the filler queue one output
                block (q/k/v = 8 accumulating matmuls + bias add) per item."""
                xt = xt_pool.tile([128, 4096], bf16, tag="xt", name="xt")
                if split:
                    for c4 in range(4):
                        nc.sync.dma_start(
                            xt[:, 1024 * c4:1024 * c4 + 1024],
                            xTb[128 * tt:128 * tt + 128,
                                1024 * c4:1024 * c4 + 1024])
                else:
                    nc.sync.dma_start(xt[:], xTb[128 * tt:128 * tt + 128, :])

                # 2-matmul items for fine pacing; a group's items stay
                # adjacent in the FIFO, so no foreign qkv-ring allocation
                # can land inside an open accumulation group
                for ob in range(3):  # q, k, v
                    ps_box = []

                    def make_item(ob, h0, ps_box=None, xt=xt):
                        def emit():
                            if h0 == 0:
                                ps_box.append(qkv_ps.tile(
                                    [128, 512], f32, tag="qkv", name="qkvp"))
                            ps = ps_box[0]
                            for hb in (h0, h0 + 1):
                                if ob < 2:
                                    w = wqk_sb[:, 256 * hb + 128 * ob:
                                               256 * hb + 128 * ob + 128]
                                else:
                                    w = wv_sb[:, 128 * hb:128 * hb + 128]
                                nc.tensor.matmul(
                                    ps[:], w, xt[:, 512 * hb:512 * hb + 512],
                                    start=(hb == 0), stop=(hb == 7))
                            if h0 == 6:
                                if ob == 0:
                                    nc.vector.tensor_scalar_add(
                                        qT_sb[:, 512 * tt:512 * tt + 512],
                                        ps[:], qkb_sb[:, 0:1])
                                elif ob == 1:
                                    nc.vector.tensor_scalar_add(
                                        kT_sb[:, 512 * tt:512 * tt + 512],
                                        ps[:], qkb_sb[:, 1:2])
                                else:
                                    nc.vector.tensor_scalar_add(
                                        vT_sb[:, 512 * tt:512 * tt + 512],
                                        ps[:], vb_sb[:, 0:1])
                        return emit

                    for h0 in (0, 2, 4, 6):
                        filler.append(
                            (430, make_item(ob, h0, ps_box),
                             ("qkv", tt) if (ob == 2 and h0 == 6) else None))

            def v_transpose(b, tbs):
                """vT [d, t] -> v slots [t, d]: PE transpose via identity,
                then per-head slot copies on the (idle) GpSimd engine."""
                for tb in tbs:
                    slot = 16 * b + tb
                    tcols = slice(2048 * b + 128 * tb, 2048 * b + 128 * tb + 128)
                    ps = qkv_ps.tile([128, 128], bf16, tag="qkv", name="vt")
                    nc.tensor.transpose(ps[:], vT_sb[:, tcols], ident_sb[:])
                    nc.scalar.copy(v_sb[:, 256 * slot:256 * slot + 64],
                                   ps[:, 0:64])
                    nc.scalar.copy(
                        v_sb[:, 256 * slot + 128:256 * slot + 192],
                        ps[:, 64:128])

            def enqueue_wo(att_sb, b, qt):
                """Partial Wo for one slice: y[tok, :] += att.T @ woT(my dims).
                Contraction = my 128 dims only; host sums partials."""
                for tb in range(4):
                    def make_item(tb, att_sb=att_sb, b=b, qt=qt):
                        def emit():
                            ps0 = qkv_ps.tile([128, 512], f32, tag="qkv",
                                              name="yps0")
                            nc.tensor.matmul(
                                ps0[:], att_sb[:, 128 * tb:128 * tb + 128],
                                wob_sb[:, 0:512], start=True, stop=True)
                            ps1 = qkv_ps.tile([128, 512], f32, tag="qkv",
                                              name="yps1")
                            nc.tensor.matmul(
                                ps1[:], att_sb[:, 128 * tb:128 * tb + 128],
                                wob_sb[:, 512:1024], start=True, stop=True)
                            ysb = y_pool.tile([128, 1024], bf16, tag="ysb",
                                              name="ysb")
                            nc.scalar.copy(ysb[:, 0:512], ps0[:])
                            nc.vector.tensor_copy(ysb[:, 512:1024], ps1[:])
                            row0 = 2048 * b + 512 * qt + 128 * tb
                            nc.gpsimd.dma_start(y_out[row0:row0 + 128, :], ysb[:])
                        return emit
                    filler.append((470, make_item(tb), None))

            def attention_slice(b, qt):
                nkb = 4 * (qt + 1)
                avA = av_ps.tile([65, 512], f32, tag="avA", name="avA")
                avB = av_ps.tile([65, 512], f32, tag="avB", name="avB")
                qlo = 2048 * b + 512 * qt
                pend = None  # AV runs one kb behind S^T/exp
                for kb in range(nkb):
                    klo = 2048 * b + 128 * kb
                    roff = kb - 4 * qt
                    lo = 128 * roff if roff > 0 else 0
                    sAB = s_ps.tile([128, 1024], f32, tag="s", name="sAB")
                    nc.tensor.matmul(
                        sAB[:, lo:512], kT_sb[0:64, klo:klo + 128],
                        qT_sb[0:64, qlo + lo:qlo + 512])
                    nc.tensor.matmul(
                        sAB[:, 512 + lo:1024], kT_sb[64:128, klo:klo + 128],
                        qT_sb[64:128, qlo + lo:qlo + 512])
                    pAB = p_pool.tile([128, 1024], bf16, tag="p", name="pAB")
                    if lo == 0:
                        nc.scalar.activation(pAB[:], sAB[:], AF.Exp,
                                             scale=1.0 / math.sqrt(HD))
                    else:
                        nc.scalar.activation(pAB[:, lo:512], sAB[:, lo:512],
                                             AF.Exp, scale=1.0 / math.sqrt(HD))
                        nc.scalar.activation(pAB[:, 512 + lo:1024],
                                             sAB[:, 512 + lo:1024],
                                             AF.Exp, scale=1.0 / math.sqrt(HD))
                    if roff >= 0:
                        c0 = 128 * roff
                        nc.vector.tensor_mul(pAB[:, c0:c0 + 128],
                                             pAB[:, c0:c0 + 128], masks_sb[:])
                        nc.vector.tensor_mul(pAB[:, 512 + c0:512 + c0 + 128],
                                             pAB[:, 512 + c0:512 + c0 + 128],
                                             masks_sb[:])
                    if pend is not None:
                        pkb, ppAB, plo = pend
                        pslot = 16 * b + pkb
                        nc.tensor.matmul(
                            avA[:, plo:512],
                            v_sb[:, 256 * pslot:256 * pslot + 65],
                            ppAB[:, plo:512], start=(pkb == 0), stop=False)
                        nc.tensor.matmul(
                            avB[:, plo:512],
                            v_sb[:, 256 * pslot + 128:256 * pslot + 193],
                            ppAB[:, 512 + plo:1024], start=(pkb == 0),
                            stop=False)
                    pend = (kb, pAB, lo)
                    drain(600)
                pkb, ppAB, plo = pend
                pslot = 16 * b + pkb
                nc.tensor.matmul(
                    avA[:, plo:512], v_sb[:, 256 * pslot:256 * pslot + 65],
                    ppAB[:, plo:512], start=(pkb == 0), stop=True)
                nc.tensor.matmul(
                    avB[:, plo:512], v_sb[:, 256 * pslot + 128:256 * pslot + 193],
                    ppAB[:, 512 + plo:1024], start=(pkb == 0), stop=True)

                # normalize: att_h = av_h[0:64] / sum_h  (sums in row 64)
                lrowA = rec_pool.tile([1, 512], f32r, tag="lrowA", name="lrowA")
                nc.vector.tensor_copy(lrowA[:], avA[64:65, :])
                lrowB = rec_pool.tile([1, 512], f32r, tag="lrowB", name="lrowB")
                nc.scalar.copy(lrowB[:], avB[64:65, :])
                rcp_ps = s_ps.tile([128, 1024], f32, tag="s", name="rcp_ps")
                nc.tensor.matmul(rcp_ps[0:64, 0:512], ones_sb[:], lrowA[:],
                                 start=True, stop=True)
                nc.tensor.matmul(rcp_ps[0:64, 512:1024], ones_sb[:], lrowB[:],
                                 start=True, stop=True)
                rcp_sb = rec_pool.tile([64, 1024], f32, tag="rcp", name="rcp_sb")
                nc.vector.reciprocal_approx_fast(rcp_sb[:], rcp_ps[0:64, :])
                att_sb = att_pool.tile([128, 512], bf16, tag="att", name="att")
                nc.vector.tensor_mul(att_sb[0:64, :], avA[0:64, :],
                                     rcp_sb[:, 0:512])
                tmpB = rec_pool.tile([64, 512], bf16, tag="tmpB", name="tmpB")
                nc.vector.tensor_mul(tmpB[:], avB[0:64, :], rcp_sb[:, 512:1024])
                nc.vector.tensor_copy(att_sb[64:128, :], tmpB[:])
                enqueue_wo(att_sb, b, qt)

            # ---------------- emission ----------------
            enqueue_qkv(0, split=True)
            drain_all()
            for qt in range(4):
                enqueue_qkv(qt + 1 if qt < 3 else 4)
                drain_until(("qkv", qt))
                v_transpose(0, range(4 * qt, 4 * qt + 4))
                attention_slice(0, qt)
            for tt in (5, 6, 7):
                enqueue_qkv(tt)
            for qt in range(4):
                drain_until(("qkv", 4 + qt))
                v_transpose(1, range(4 * qt, 4 * qt + 4))
                attention_slice(1, qt)
            drain_all()
        ctx_lp.__exit__(None, None, None)

    nc.compile()
    return nc


def host_prep(x, Wqkv_w, Wqkv_b, Wo_w, Wo_b):
    import ml_dtypes
    bf16 = ml_dtypes.bfloat16

    x = np.asarray(x, np.float32)
    Wqkv_w = np.asarray(Wqkv_w, np.float32)
    Wqkv_b = np.asarray(Wqkv_b, np.float32)
    Wo_w = np.asarray(Wo_w, np.float32)

    xf = x.reshape(4096, 1024)
    xTb = np.ascontiguousarray(
        xf.reshape(8, 512, 8, 128).transpose(0, 3, 2, 1).reshape(1024, 4096)
    ).astype(bf16)
    masks = np.ascontiguousarray(
        (np.arange(128)[:, None] <= np.arange(128)[None, :])
    ).astype(bf16)
    ones1x64 = np.ones((1, 64), dtype=np.float32)
    identm = np.ascontiguousarray(np.eye(128, dtype=np.float32)).astype(bf16)

    in_maps = []
    for i in range(NCORES):
        hA, hB = 2 * i, 2 * i + 1
        rows_qk = np.r_[64 * hA:64 * hA + 64, 64 * hB:64 * hB + 64,
                        1024 + 64 * hA:1024 + 64 * hA + 64,
                        1024 + 64 * hB:1024 + 64 * hB + 64]
        Wsub = Wqkv_w[rows_qk]                      # [256, 1024]
        wqkb = np.ascontiguousarray(
            Wsub.reshape(2, 128, 8, 128).transpose(3, 2, 0, 1).reshape(128, 2048)
        ).astype(bf16)
        qkbias = np.ascontiguousarray(Wqkv_b[rows_qk].reshape(2, 128).T)
        rows_v = np.r_[2048 + 64 * hA:2048 + 64 * hA + 64,
                       2048 + 64 * hB:2048 + 64 * hB + 64]
        Vsub = Wqkv_w[rows_v]                       # [128, 1024]
        wvb = np.ascontiguousarray(
            Vsub.reshape(128, 8, 128).transpose(2, 1, 0).reshape(128, 1024)
        ).astype(bf16)
        vbcol = np.ascontiguousarray(Wqkv_b[rows_v].reshape(128, 1))
        rows_o = np.r_[64 * hA:64 * hA + 64, 64 * hB:64 * hB + 64]
        wob = np.ascontiguousarray(Wo_w[:, rows_o].T).astype(bf16)
        in_maps.append(dict(
            xTb=xTb, wqkb=wqkb, qkb=qkbias, wvb=wvb, vbcol=vbcol,
            masks128=masks, ones1x64=ones1x64, wob=wob, ident=identm))
    return in_maps


def _ensure_ntff_hook_module():
    """run_bass_kernel_spmd(trace=True) under axon imports
    antenv.axon_hooks; provide a ctypes-based fallback if absent."""
    import importlib
    import sys
    import types
    try:
        importlib.import_module("antenv.axon_hooks")
        return
    except ImportError:
        pass
    import contextlib
    import ctypes

    mod = types.ModuleType("antenv.axon_hooks")
    state = {"hook": None}

    def set_axon_ntff_profile_hook(h):
        state["hook"] = h

    def _make():
        try:
            lib = ctypes.CDLL("/opt/axon/libaxon_pjrt.so")
        except OSError:
            return None
        if not hasattr(lib, "axon_start_nrt_profile"):
            return None
        lib.axon_start_nrt_profile.argtypes = [
            ctypes.POINTER(ctypes.c_int64), ctypes.c_size_t]
        lib.axon_start_nrt_profile.restype = ctypes.c_int64
        lib.axon_stop_nrt_profile.argtypes = [ctypes.c_char_p]
        lib.axon_stop_nrt_profile.restype = ctypes.c_int64

        @contextlib.contextmanager
        def _hook(output_dir, device_ids):
            import jax
            jax.devices()
            if device_ids:
                ids = (ctypes.c_int64 * len(device_ids))(*device_ids)
                rc = lib.axon_start_nrt_profile(ids, len(device_ids))
            else:
                rc = lib.axon_start_nrt_profile(None, 0)
            if rc != 0:
                raise RuntimeError(f"axon_start_nrt_profile rc={rc}")
            try:
                yield
            finally:
                lib.axon_stop_nrt_profile(str(output_dir).encode())

        return _hook

    def get_axon_ntff_profile_hook():
        if state["hook"] is None:
            state["hook"] = _make()
        return state["hook"]

    mod.set_axon_ntff_profile_hook = set_axon_ntff_profile_hook
    mod.get_axon_ntff_profile_hook = get_axon_ntff_profile_hook
    try:
        import antenv
        sys.modules["antenv.axon_hooks"] = mod
        antenv.axon_hooks = mod
    except ImportError:
        pass


def kernel(x, Wqkv_w, Wqkv_b, Wo_w, Wo_b):
    from concourse import bass_utils

    _ensure_ntff_hook_module()

    if "nc" not in _cache:
        _cache["nc"] = _build()
    nc = _cache["nc"]

    in_maps = host_prep(x, Wqkv_w, Wqkv_b, Wo_w, Wo_b)
    res = bass_utils.run_bass_kernel_spmd(nc, in_maps, core_ids=list(range(NCORES)))
    _cache["last_results"] = res

    acc = np.zeros((4096, 1024), np.float32)
    for i in range(NCORES):
        acc += res.results[i]["y"].astype(np.float32)
    acc += np.asarray(Wo_b, np.float32)[None, :]
    return acc.reshape(B, C, H)

